# revision 1
# baseline (speedup 1.0000x reference)
"""Trainium2 Bass kernel for CombinedPriorityLoss (MSE + pairwise ranking + diversity).

Strategy: shard the 8192x8192 pairwise matrix by rows across 8 cores
(1024 rows each). Each core computes partial sums of the masked pairwise
ranking loss for its row-slab against the full column vector, plus
partial O(N) stats for the MSE/variance terms. Host combines scalars.

Math (all-pairs symmetric form; m = MARGIN):
  per ordered pair (i,j): dp = p_i - p_j, dt = t_i - t_j
    f = 1[dt>m]*relu(m-dp) + 1[dt<-m]*relu(m+dp) + 1[|dt|<=m]*0.1*|dp|
  f is symmetric under (i,j)<->(j,i), diagonal is 0, so
    sum_{i<j} f = 0.5 * sum_{all i,j} f
  and the lo-branch maps onto the hi-branch under the swap:
    sum_all f = sum_all [ c1*(2r - v) + cle*v ]
  with c1 = 1[t_j < t_i - m], cle = 1[t_j <= t_i + m],
       r = relu(m - dp), v = 0.1*|dp|.
"""

import numpy as np

import concourse.bacc as bacc
import concourse.mybir as mybir
from concourse.tile import TileContext
from concourse.bass_utils import run_bass_kernel_spmd

N = 8192
N_CORES = 8
ROWS_PER_CORE = N // N_CORES          # 1024
RB = ROWS_PER_CORE // 128             # 8 row blocks per core
MARGIN = 0.2
MSE_W = 0.1
RANK_W = 0.9
DIV_W = 0.1

F32 = mybir.dt.float32
Alu = mybir.AluOpType
Act = mybir.ActivationFunctionType


def _build(reps: int = 1, ct: int = 2048):
    """Build the per-core Bass program. Returns compiled nc."""
    n_ct = N // ct
    nacc = RB * n_ct

    nc = bacc.Bacc(None)
    pcol = nc.dram_tensor("pcol", [N], F32, kind="ExternalInput")
    tcol = nc.dram_tensor("tcol", [N], F32, kind="ExternalInput")
    prow = nc.dram_tensor("prow", [ROWS_PER_CORE], F32, kind="ExternalInput")
    trow = nc.dram_tensor("trow", [ROWS_PER_CORE], F32, kind="ExternalInput")
    accA_d = nc.dram_tensor("accA", [128, nacc], F32, kind="ExternalOutput")
    accB_d = nc.dram_tensor("accB", [128, nacc], F32, kind="ExternalOutput")
    stats_d = nc.dram_tensor("stats", [128, 5], F32, kind="ExternalOutput")

    with TileContext(nc) as tc:
        with (
            tc.tile_pool(name="bcast", bufs=1) as bpool,
            tc.tile_pool(name="rows", bufs=1) as rpool,
            tc.tile_pool(name="work", bufs=2) as wpool,
            tc.tile_pool(name="accs", bufs=1) as apool,
        ):
            # --- broadcast column tiles (full vectors along free dim) ---
            pcol_b = bpool.tile([128, N], F32, name="pcol_b")
            tcol_b = bpool.tile([128, N], F32, name="tcol_b")
            nchunk = 4
            for i in range(nchunk):
                sl = slice(i * (N // nchunk), (i + 1) * (N // nchunk))
                nc.sync.dma_start(pcol_b[:, sl], pcol[None, sl].partition_broadcast(128))
                nc.sync.dma_start(tcol_b[:, sl], tcol[None, sl].partition_broadcast(128))

            # --- row scalars: [128, RB] (partition = row-in-block, free = rb) ---
            prow_t = rpool.tile([128, RB], F32, name="prow_t")
            trow_t = rpool.tile([128, RB], F32, name="trow_t")
            nc.sync.dma_start(prow_t[:, :], prow.rearrange("(rb p) -> p rb", p=128))
            nc.sync.dma_start(trow_t[:, :], trow.rearrange("(rb p) -> p rb", p=128))

            # per-rb per-partition scalars
            t_lo = rpool.tile([128, RB], F32, name="t_lo")     # t_row - m
            t_hi = rpool.tile([128, RB], F32, name="t_hi")     # t_row + m
            r_bias = rpool.tile([128, RB], F32, name="r_bias")  # m - p_row
            v_bias = rpool.tile([128, RB], F32, name="v_bias")  # -0.1 * p_row
            nc.vector.tensor_scalar(t_lo[:, :], trow_t[:, :], -MARGIN, None, Alu.add)
            nc.vector.tensor_scalar(t_hi[:, :], trow_t[:, :], MARGIN, None, Alu.add)
            nc.vector.tensor_scalar(r_bias[:, :], prow_t[:, :], -1.0, MARGIN, Alu.mult, Alu.add)
            nc.vector.tensor_scalar(v_bias[:, :], prow_t[:, :], -0.1, None, Alu.mult)

            # --- O(N) stats on this core's row slice ---
            stats_t = apool.tile([128, 5], F32, name="stats_t")
            d_t = rpool.tile([128, RB], F32, name="d_t")
            nc.vector.scalar_tensor_tensor(d_t[:, :], prow_t[:, :], 1.0, trow_t[:, :],
                                           Alu.mult, Alu.subtract)
            scr = rpool.tile([128, RB], F32, name="scr")
            nc.vector.scalar_tensor_tensor(scr[:, :], d_t[:, :], 1.0, d_t[:, :],
                                           Alu.mult, Alu.mult, accum_out=stats_t[:, 0:1])
            nc.vector.scalar_tensor_tensor(scr[:, :], prow_t[:, :], 1.0, prow_t[:, :],
                                           Alu.mult, Alu.mult, accum_out=stats_t[:, 1:2])
            nc.vector.scalar_tensor_tensor(scr[:, :], trow_t[:, :], 1.0, trow_t[:, :],
                                           Alu.mult, Alu.mult, accum_out=stats_t[:, 2:3])
            nc.vector.tensor_scalar(scr[:, :], prow_t[:, :], 1.0, 0.0, Alu.mult, Alu.add,
                                    accum_out=stats_t[:, 3:4])
            nc.vector.tensor_scalar(scr[:, :], trow_t[:, :], 1.0, 0.0, Alu.mult, Alu.add,
                                    accum_out=stats_t[:, 4:5])

            # --- main pairwise loop ---
            accA = apool.tile([128, nacc], F32, name="accA_t")
            accB = apool.tile([128, nacc], F32, name="accB_t")
            for _rep in range(reps):
                for rb in range(RB):
                    for ci in range(n_ct):
                        cs = slice(ci * ct, (ci + 1) * ct)
                        idx = rb * n_ct + ci
                        c1 = wpool.tile([128, ct], F32, name="c1")
                        cle = wpool.tile([128, ct], F32, name="cle")
                        r = wpool.tile([128, ct], F32, name="r")
                        v = wpool.tile([128, ct], F32, name="v")
                        w = wpool.tile([128, ct], F32, name="w")
                        # masks on DVE
                        nc.vector.tensor_scalar(c1[:, :], tcol_b[:, cs],
                                                t_lo[:, rb:rb + 1], None, Alu.is_lt)
                        nc.vector.tensor_scalar(cle[:, :], tcol_b[:, cs],
                                                t_hi[:, rb:rb + 1], None, Alu.is_le)
                        # branch values on ACT
                        nc.scalar.activation(r[:, :], pcol_b[:, cs], Act.Relu,
                                             bias=r_bias[:, rb:rb + 1], scale=1.0)
                        nc.scalar.activation(v[:, :], pcol_b[:, cs], Act.Abs,
                                             bias=v_bias[:, rb:rb + 1], scale=0.1)
                        # w = 2r - v ; accumulate c1*w and cle*v
                        nc.vector.scalar_tensor_tensor(w[:, :], r[:, :], 2.0, v[:, :],
                                                       Alu.mult, Alu.subtract)
                        nc.vector.scalar_tensor_tensor(
                            w[:, :], c1[:, :], 1.0, w[:, :], Alu.mult, Alu.mult,
                            accum_out=accA[:, idx:idx + 1])
                        nc.vector.scalar_tensor_tensor(
                            v[:, :], cle[:, :], 1.0, v[:, :], Alu.mult, Alu.mult,
                            accum_out=accB[:, idx:idx + 1])

            nc.sync.dma_start(accA_d[:, :], accA[:, :])
            nc.sync.dma_start(accB_d[:, :], accB[:, :])
            nc.sync.dma_start(stats_d[:, :], stats_t[:, :])

    nc.compile()
    return nc


_NC_CACHE = {}


def _get_nc(reps: int = 1):
    key = reps
    if key not in _NC_CACHE:
        _NC_CACHE[key] = _build(reps=reps)
    return _NC_CACHE[key]


class _CachedRunner:
    """Build the shard_map-jitted bass_exec callable once, reuse across calls.

    run_bass_kernel_spmd -> run_bass_via_pjrt constructs a fresh closure and
    jax.jit on every invocation (full retrace each call); this caches it.
    """

    def __init__(self, nc):
        import jax
        from jax.experimental.shard_map import shard_map
        from jax.sharding import Mesh, PartitionSpec
        from concourse import bass2jax, mybir as _mybir

        bass2jax.install_neuronx_cc_hook()
        self.nc = nc
        in_names, out_names, out_avals = [], [], []
        partition_name = (nc.partition_id_tensor.name
                          if nc.partition_id_tensor else None)
        for alloc in nc.m.functions[0].allocations:
            if not isinstance(alloc, _mybir.MemoryLocationSet):
                continue
            name = alloc.memorylocations[0].name
            if alloc.kind == "ExternalInput":
                if name != partition_name:
                    in_names.append(name)
            elif alloc.kind == "ExternalOutput":
                out_avals.append(jax.core.ShapedArray(
                    tuple(alloc.tensor_shape), _mybir.dt.np(alloc.dtype)))
                out_names.append(name)
        self.in_names, self.out_names, self.out_avals = in_names, out_names, out_avals
        n_params, n_outs = len(in_names), len(out_names)
        self.n_params = n_params
        all_names = in_names + out_names + ([partition_name] if partition_name else [])

        def _body(*args):
            operands = list(args)
            if partition_name is not None:
                operands.append(bass2jax.partition_id_tensor())
            return tuple(bass2jax._bass_exec_p.bind(
                *operands,
                out_avals=tuple(out_avals),
                in_names=tuple(all_names),
                out_names=tuple(out_names),
                lowering_input_output_aliases=(),
                sim_require_finite=True,
                sim_require_nnan=True,
                nc=nc,
            ))

        devices = jax.devices()[:N_CORES]
        mesh = Mesh(np.asarray(devices), ("core",))
        in_specs = (PartitionSpec("core"),) * (n_params + n_outs)
        out_specs = (PartitionSpec("core"),) * n_outs
        self.fn = jax.jit(
            shard_map(_body, mesh=mesh, in_specs=in_specs, out_specs=out_specs,
                      check_rep=False),
            donate_argnums=tuple(range(n_params, n_params + n_outs)),
            keep_unused=True,
        )

    def __call__(self, in_maps):
        concat_in = [
            np.concatenate([np.asarray(m[name]) for m in in_maps], axis=0)
            for name in self.in_names
        ]
        concat_zeros = [
            np.zeros((N_CORES * a.shape[0], *a.shape[1:]), a.dtype)
            for a in self.out_avals
        ]
        out_arrs = self.fn(*concat_in, *concat_zeros)
        import jax
        jax.block_until_ready(out_arrs)
        return [
            {name: np.asarray(out_arrs[i]).reshape(
                N_CORES, *self.out_avals[i].shape)[c]
             for i, name in enumerate(self.out_names)}
            for c in range(N_CORES)
        ]


_RUNNER_CACHE = {}


def _get_runner(reps: int = 1):
    if reps not in _RUNNER_CACHE:
        _RUNNER_CACHE[reps] = _CachedRunner(_get_nc(reps))
    return _RUNNER_CACHE[reps]


def _in_maps(p: np.ndarray, t: np.ndarray):
    in_maps = []
    for c in range(N_CORES):
        rs = slice(c * ROWS_PER_CORE, (c + 1) * ROWS_PER_CORE)
        in_maps.append({
            "pcol": p, "tcol": t,
            "prow": np.ascontiguousarray(p[rs]),
            "trow": np.ascontiguousarray(t[rs]),
        })
    return in_maps


def _run(nc, p: np.ndarray, t: np.ndarray):
    return run_bass_kernel_spmd(nc, _in_maps(p, t), core_ids=list(range(N_CORES)))


def _combine(results) -> np.float32:
    A = 0.0
    B = 0.0
    s_d2 = s_p2 = s_t2 = s_p = s_t = 0.0
    for r in results:
        A += float(r["accA"].astype(np.float64).sum())
        B += float(r["accB"].astype(np.float64).sum())
        st = r["stats"].astype(np.float64)
        s_d2 += st[:, 0].sum()
        s_p2 += st[:, 1].sum()
        s_t2 += st[:, 2].sum()
        s_p += st[:, 3].sum()
        s_t += st[:, 4].sum()

    pair_count = N * (N - 1) // 2
    rank = 0.5 * (A + B) / pair_count
    mse = s_d2 / N
    var_p = (s_p2 - s_p * s_p / N) / (N - 1)
    var_t = (s_t2 - s_t * s_t / N) / (N - 1)
    div = max(var_t - var_p, 0.0)
    return np.float32(MSE_W * mse + RANK_W * rank + DIV_W * div)


def kernel(predictions, targets) -> np.ndarray:
    p = np.asarray(predictions, dtype=np.float32)
    t = np.asarray(targets, dtype=np.float32)
    runner = _get_runner(reps=1)
    results = runner(_in_maps(p, t))
    out = _combine(results)
    return np.asarray(out, dtype=np.float32)

